# revision 19
# baseline (speedup 1.0000x reference)
"""Polynomial flow regularizer loss on 8 Trainium2 NeuronCores — fp8 version.

reference semantics: fit a quadratic polynomial surface (basis
[1, x, y, x^2, x*y, y^2] over a [-1,1]^2 grid) to each (b, c) image of
flow_field (64, 2, 512, 512) via least squares, and return
mean_b(sum_c(mean_pixels((f - fit)^2))).

Math: with Phi the (N, 6) basis, G = Phi^T Phi and r = Phi^T f, the
residual energy is ||f||^2 - r^T G^-1 r.  The basis separates in (x, y),
so r is recoverable on host from V[a, w] = sum_h y_h^a f[h, w] (a=0,1,2).

fp8 design (vs the bf16 baseline).  The chip power-throttles when all
engines run hot (≈50% util cap for half the kernel), so total
engine-seconds is the currency:
  - host pre-casts to fp8 e4m3 (TRN flavor): 4.19MB per core, half the
    bf16 HBM stream.  Loss bias ~-8e-4 relative (tolerance 2e-2).
  - V via DoubleRow fp8 matmuls (2 elems/lane/cycle): contraction pairs
    the two 256-row halves; one matmul per (image, row-parity).  Weight
    tile zero-padded 3->16 columns (DoubleRow LDWEIGHTS ISA minimum).
    Row layout h = 256*cp + 2p + t keeps DMA lines 1KB contiguous.
  - V outputs stripe-packed into ONE psum bank per group by ROTATING the
    basis inside the weight tile: image j's basis sits at weight columns
    3j..3j+2 (zeros elsewhere), every matmul accumulates into the same
    (16, 512) block at base partition 0 (the ISA rejects DoubleRow dst
    offsets), so the mandatory PSUM->SBUF copy is one op over 512 free
    elems per group, engine-alternated, instead of n*512.
  - ||f||^2: one pass per engine with hardware accumulators: ACT
    activation(Square, accum_out) on w[0:254], DVE scalar_tensor_tensor
    (x*1*x, accum_out) on w[254:512]; fp8 scratch outs halve SBUF write
    power.  Pool's mul measured 3.2 ns/elem (vs ACT 0.90 / DVE 1.08) —
    worse than useless under the power cap, so Pool only runs one DMA
    ring.  Routing square-reductions through spare PE cycles via
    host-squared fp8 + ones-weight matmuls measured SLOWER (the extra
    stream delays the image DMAs on the shared queues).
  - input stream alternates between the sync HWDGE ring and the gpsimd
    SWDGE ring (~220 GB/s each); first group is small so compute starts
    early.
Host work: r assembly from V, the 6x6 solve, final reduction.
"""

import sys

import numpy as np

sys.path.insert(0, "/opt/trn_rl_repo")

import concourse.bacc as bacc
import concourse.bass as bass
import concourse.tile as tile
from concourse import mybir
from concourse.bass_utils import run_bass_kernel_spmd

B, C, H, W = 64, 2, 512, 512
N_CORES = 8
IMGS = (B // N_CORES) * C  # images per core = 16
F32 = mybir.dt.float32
FP8 = mybir.dt.float8e4
BF16 = mybir.dt.bfloat16

GROUPS = [1, 2, 3, 3, 3, 3, 1]
NG = len(GROUPS)
# w-column split of the squares across ACT / DVE (measured 0.90 / 1.08
# ns/elem incl. overheads); both engines also alternate the V copy.
# engine squares cover w[0:WQTOT] (the PE reduces host-shipped f^2 for
# w[WQTOT:512], except in the last group); split by measured rates
WA, WD = 190, 194
WQTOT = WA + WD  # 384
WQ = W - WQTOT  # 128 columns on the PE
# last group (after the sq-matmul chain stops): engines cover all 512
WAL, WDL = 254, 258

_NC = None


def _pn(n):
    """partitions used by n stripes: image j at partitions 3j..3j+2."""
    return 3 * n


def _build():
    nc = bacc.Bacc()
    flow = nc.declare_dram_parameter("flow", [IMGS, H, W], FP8, isOutput=False)
    # host-squared fp8 of w[384:512], pre-arranged in the SBUF tile layout
    # [p, i, cp, t, w] (rows h = 256*cp + 2p + t)
    flowsq = nc.declare_dram_parameter(
        "flowsq", [128, 2, 2, IMGS, WQ], FP8, isOutput=False
    )
    onesw = nc.declare_dram_parameter("onesw", [128, 2, 16], FP8, isOutput=False)
    # ybasis[k, t, cp, m] = (y at row h=256*cp+2k+t) ** m, fp8-rounded,
    # ybasis[k, j, t, cp, m]: weight set for group-image j, parity t:
    # basis value at column m=3j+a, zeros elsewhere (m padded to 16,
    # the DoubleRow LDWEIGHTS ISA minimum)
    ybas = nc.declare_dram_parameter(
        "ybasis", [128, 3, 2, 2, 16], FP8, isOutput=False
    )
    # v_out[3j+a, g, w] = V[a, image g0+j, w]
    v_out = nc.declare_dram_parameter("v_out", [16, NG, W], BF16, isOutput=True)
    sq_out = nc.declare_dram_parameter("sq_out", [128, 2, NG], F32, isOutput=True)
    sqm_out = nc.declare_dram_parameter("sqm_out", [16, 3 * WQ], BF16, isOutput=True)

    with tile.TileContext(nc) as tc:
        with (
            tc.tile_pool(name="const", bufs=1) as cpool,
            tc.tile_pool(name="img", bufs=7) as ipool,
            tc.tile_pool(name="scr", bufs=2) as spool,
            tc.tile_pool(name="psum", bufs=4, space="PSUM") as ppool,
        ):
            yb = cpool.tile([128, 3, 2, 2, 16], FP8)
            nc.scalar.dma_start(out=yb[:], in_=ybas[:])
            ow = cpool.tile([128, 2, 16], FP8)
            nc.scalar.dma_start(out=ow[:], in_=onesw[:])
            # all groups' PE-side square sums accumulate here; column
            # j*WQ+c collects image-slot j of every group
            psq = pqpool.tile([16, 3 * WQ], F32)
            # dummy Square so the ACT table load (~1.3us) happens in the
            # preamble while ACT is idle, not before the first real square
            warm = cpool.tile([128, 1], F32)
            nc.scalar.activation(
                out=warm[:],
                in_=nc.const_aps.scalar_like(1.0, warm[:]),
                func=mybir.ActivationFunctionType.Square,
            )
            # one tile for both engines' accumulators -> one output DMA
            sq_ad = cpool.tile([128, 2, NG], F32)
            # all groups' V stripes stage here; ONE output DMA at the end
            # (per-group DMAs on the sync queue block later input DMAs)
            v_all = cpool.tile([16, NG, W], BF16)

            g0 = 0
            for g, n in enumerate(GROUPS):
                # img[p, i, cp, t, w]: row h = 256*cp + 2p + t of image i.
                # (t, w) is 1KB contiguous on both sides; (i, cp) merge on
                # the DRAM side -> 3-dim full-rate DMA pattern.
                img = ipool.tile([128, n, 2, 2, W], FP8, tag="img")
                # sync's queue delivers first data ~1us before gpsimd's:
                # route the pipeline-filling first two groups there
                dma_eng = nc.sync if g in (0, 1, 3, 5) else nc.gpsimd
                dma_eng.dma_start(
                    out=img[:],
                    in_=flow[g0 : g0 + n].rearrange(
                        "i (cp p t) w -> p i cp (t w)", cp=2, p=128, t=2
                    ),
                )

                # V: per (image, parity) one DoubleRow matmul contracting
                # both cp halves; image j's weight set routes its basis to
                # output rows 3j..3j+2, zeros elsewhere, so all 2n matmuls
                # accumulate into one shared (16, W) block.
                psum = ppool.tile([16, W], F32, tag="v")
                for j in range(n):
                    for t in range(2):
                        nc.tensor.matmul(
                            psum[:],
                            yb[:, j, t, :, :],
                            img[:, j, :, t, :],
                            start=(j == 0 and t == 0),
                            stop=(j == n - 1 and t == 1),
                            perf_mode=mybir.MatmulPerfMode.DoubleRow,
                        )

                pe_sq = n == 3 and g <= NG - 2
                if pe_sq:
                    # PE-side squares: ones-weight DoubleRow matmuls reduce
                    # the host-squared stream over h, accumulating across
                    # the n=3 groups in one full-width chain
                    sqi = ipool.tile([128, 2, 2, n, WQ], FP8, tag="sqi")
                    dma_eng.dma_start(
                        out=sqi[:], in_=flowsq[:, :, :, g0 : g0 + n, :]
                    )
                    for t in range(2):
                        nc.tensor.matmul(
                            psq[:],
                            ow[:],
                            sqi[:, :, t, :, :].rearrange("p cp i w -> p cp (i w)"),
                            start=(g == 2 and t == 0),
                            stop=(g == NG - 2 and t == 1),
                            perf_mode=mybir.MatmulPerfMode.DoubleRow,
                        )

                # squares: one pass per engine, hardware accumulators.
                # fp8 scratch: the result stream is discarded, and fp8 out
                # halves SBUF write traffic (the chip power-throttles).
                # last group: DVE (slower rate) gets fewer columns so both
                # engines finish the tail together
                wa, wd = (WA, WD) if g < NG - 1 else (300, 212)
                scr_a = spool.tile([128, 3, 2, 2, WAL], FP8, tag="scra")
                nc.scalar.activation(
                    out=scr_a[:, :n, :, :, 0:wa],
                    in_=img[:, :, :, :, 0:wa],
                    func=mybir.ActivationFunctionType.Square,
                    accum_out=sq_ad[:, 0, g : g + 1],
                )
                scr_d = spool.tile([128, 3, 2, 2, WDL], FP8, tag="scrd")
                nc.vector.scalar_tensor_tensor(
                    out=scr_d[:, :n, :, :, 0:wd],
                    in0=img[:, :, :, :, wa : wa + wd],
                    scalar=1.0,
                    in1=img[:, :, :, :, wa : wa + wd],
                    op0=mybir.AluOpType.mult,
                    op1=mybir.AluOpType.mult,
                    accum_out=sq_ad[:, 1, g : g + 1],
                )
                if g == NG - 2:
                    # sq chain just stopped: exit it before the last group's
                    # engine squares so the DMA overlaps the tail
                    sqm_sb = cpool.tile([16, 3 * WQ], BF16)
                    nc.scalar.copy(out=sqm_sb[:], in_=psq[:])
                    nc.sync.dma_start(out=sqm_out[:], in_=sqm_sb[:])

                # V exit: one bf16 staging copy over the whole stripe block
                # (free size 512 regardless of n), engines alternating.
                if g % 2 == 0:
                    nc.scalar.copy(out=v_all[:, g, :], in_=psum[:])
                else:
                    nc.vector.tensor_copy(out=v_all[:, g, :], in_=psum[:])
                if g == NG - 1:
                    # exit DMAs, emitted after the last input DMA so their
                    # semaphore waits cannot block it; groups 0..NG-2's V
                    # flushes while the last group still computes
                    nc.sync.dma_start(
                        out=v_out[:, 0 : NG - 1, :], in_=v_all[:, 0 : NG - 1, :]
                    )
                    nc.sync.dma_start(
                        out=v_out[:, NG - 1 :, :], in_=v_all[:, NG - 1 :, :]
                    )
                    nc.sync.dma_start(out=sq_out[:], in_=sq_ad[:])
                g0 += n


    nc.finalize()
    return nc


def _yvals():
    """(q(y), q(y*y)) on the fp8 e4m3 grid, f64."""
    import ml_dtypes

    y = np.linspace(-1.0, 1.0, H, dtype=np.float32)
    qy = y.astype(ml_dtypes.float8_e4m3).astype(np.float64)
    qy2 = (y * y).astype(ml_dtypes.float8_e4m3).astype(np.float64)
    return qy, qy2


def _onesw():
    import ml_dtypes

    O = np.zeros((128, 2, 16), dtype=np.float64)
    O[:, :, 0] = 1.0
    return O.astype(ml_dtypes.float8_e4m3)


def _flowsq(shard8):
    """shard8: (IMGS, H, W) fp8 -> (128, IMGS, 2, 2, WQ) fp8 of f^2 for
    w[WQTOT:], laid out as the SBUF tile [p, i, cp, t, w]."""
    import ml_dtypes

    f2 = (shard8.astype(np.float32) ** 2)[:, :, WQTOT:]
    f2 = f2.reshape(IMGS, 2, 128, 2, WQ)  # i, cp, p, t, w
    return np.ascontiguousarray(
        f2.transpose(2, 1, 3, 0, 4).astype(ml_dtypes.float8_e4m3)
    )


def _ybasis():
    import ml_dtypes

    qy, qy2 = _yvals()
    Y = np.zeros((128, 3, 2, 2, 16), dtype=np.float64)
    for j in range(3):
        for t in range(2):
            for cp in range(2):
                h = 256 * cp + 2 * np.arange(128) + t
                Y[:, j, t, cp, 3 * j + 0] = 1.0
                Y[:, j, t, cp, 3 * j + 1] = qy[h]
                Y[:, j, t, cp, 3 * j + 2] = qy2[h]
    return Y.astype(ml_dtypes.float8_e4m3)


def _gram():
    # G = Phi^T Phi for the basis the device actually applies: y-side on the
    # fp8 grid, x-side exact f64.  Each entry factorizes into y-sum * x-sum.
    qy, qy2 = _yvals()
    yv = [np.ones_like(qy), qy, qy2]
    x = np.linspace(-1.0, 1.0, W, dtype=np.float32).astype(np.float64)
    xv = [np.ones_like(x), x, x * x]
    e = [(0, 0), (0, 1), (1, 0), (0, 2), (1, 1), (2, 0)]
    G = np.empty((6, 6))
    for j in range(6):
        for k in range(6):
            G[j, k] = (yv[e[j][0]] * yv[e[k][0]]).sum() * (
                xv[e[j][1]] * xv[e[k][1]]
            ).sum()
    return G


def _extract_v(v_raw):
    """v_raw: (16, NG, W) -> V (3, IMGS, W) f64."""
    V = np.empty((3, IMGS, W), dtype=np.float64)
    g0 = 0
    for g, n in enumerate(GROUPS):
        for j in range(n):
            for a in range(3):
                V[a, g0 + j, :] = v_raw[3 * j + a, g, :]
        g0 += n
    return V


def _run(shards, ybasis=None, trace=False, **kwargs):
    """shards: (8, IMGS, H, W) float32-or-fp8. Returns BassKernelResults."""
    import ml_dtypes

    global _NC
    if _NC is None:
        _NC = _build()
    if ybasis is None:
        ybasis = _ybasis()
    shards = np.asarray(shards)
    if shards.dtype != ml_dtypes.float8_e4m3:
        shards = shards.astype(ml_dtypes.float8_e4m3)
    in_maps = [
        {"flow": np.ascontiguousarray(shards[k]), "ybasis": ybasis}
        for k in range(N_CORES)
    ]
    return run_bass_kernel_spmd(_NC, in_maps, list(range(N_CORES)), trace=trace, **kwargs)


def kernel(flow_field: np.ndarray) -> np.ndarray:
    import ml_dtypes

    global _NC
    flow = np.asarray(flow_field, dtype=np.float32)
    assert flow.shape == (B, C, H, W)
    shards = np.ascontiguousarray(
        flow.reshape(N_CORES, IMGS, H, W).astype(ml_dtypes.float8_e4m3)
    )

    # rare transient NRT device errors recover on a clean retry
    last_err = None
    for attempt in range(3):
        try:
            res = _run(shards)
            break
        except Exception as e:  # noqa: BLE001
            last_err = e
            _NC = None
    else:
        raise last_err

    G = _gram()
    x = np.linspace(-1.0, 1.0, W, dtype=np.float32).astype(np.float64)
    Xb = np.stack([np.ones_like(x), x, x * x], axis=1)  # (W, 3)

    Ginv = np.linalg.inv(G)
    total = 0.0
    for k in range(N_CORES):
        v = _extract_v(np.asarray(res.results[k]["v_out"], dtype=np.float64))
        sq = np.asarray(res.results[k]["sq_out"], dtype=np.float64)  # (128, 2, NG)
        sqm = np.asarray(res.results[k]["sqm_out"], dtype=np.float64)
        M = np.einsum("aiw,wb->iab", v, Xb)  # (IMGS, 3, 3)
        r = np.stack(
            [M[:, 0, 0], M[:, 0, 1], M[:, 1, 0], M[:, 0, 2], M[:, 1, 1], M[:, 2, 0]],
            axis=1,
        )  # (IMGS, 6)
        fit_energy = np.einsum("ij,jk,ik->i", r, Ginv, r)  # r^T G^-1 r
        total += float(sq.sum() - fit_energy.sum())

    loss = total / (H * W) / B
    return np.asarray(loss, dtype=np.float32)


# revision 20
# speedup vs baseline: 1.2084x; 1.2084x over previous
"""Polynomial flow regularizer loss on 8 Trainium2 NeuronCores — fp8 version.

reference semantics: fit a quadratic polynomial surface (basis
[1, x, y, x^2, x*y, y^2] over a [-1,1]^2 grid) to each (b, c) image of
flow_field (64, 2, 512, 512) via least squares, and return
mean_b(sum_c(mean_pixels((f - fit)^2))).

Math: with Phi the (N, 6) basis, G = Phi^T Phi and r = Phi^T f, the
residual energy is ||f||^2 - r^T G^-1 r.  The basis separates in (x, y),
so r is recoverable on host from V[a, w] = sum_h y_h^a f[h, w] (a=0,1,2).

fp8 design (vs the bf16 baseline).  The chip power-throttles when all
engines run hot (≈50% util cap for half the kernel), so total
engine-seconds is the currency:
  - host pre-casts to fp8 e4m3 (TRN flavor): 4.19MB per core, half the
    bf16 HBM stream.  Loss bias ~-8e-4 relative (tolerance 2e-2).
  - V via DoubleRow fp8 matmuls (2 elems/lane/cycle): contraction pairs
    the two 256-row halves; one matmul per (image, row-parity).  Weight
    tile zero-padded 3->16 columns (DoubleRow LDWEIGHTS ISA minimum).
    Row layout h = 256*cp + 2p + t keeps DMA lines 1KB contiguous.
  - V outputs stripe-packed into ONE psum bank per group by ROTATING the
    basis inside the weight tile: image j's basis sits at weight columns
    3j..3j+2 (zeros elsewhere), every matmul accumulates into the same
    (16, 512) block at base partition 0 (the ISA rejects DoubleRow dst
    offsets), so the mandatory PSUM->SBUF copy is one op over 512 free
    elems per group, engine-alternated, instead of n*512.
  - ||f||^2: one pass per engine with hardware accumulators: ACT
    activation(Square, accum_out) on w[0:254], DVE scalar_tensor_tensor
    (x*1*x, accum_out) on w[254:512]; fp8 scratch outs halve SBUF write
    power.  Pool's mul measured 3.2 ns/elem (vs ACT 0.90 / DVE 1.08) —
    worse than useless under the power cap, so Pool only runs one DMA
    ring.  Routing square-reductions through spare PE cycles via
    host-squared fp8 + ones-weight matmuls measured SLOWER (the extra
    stream delays the image DMAs on the shared queues).
  - input stream alternates between the sync HWDGE ring and the gpsimd
    SWDGE ring (~220 GB/s each); first group is small so compute starts
    early.
Host work: r assembly from V, the 6x6 solve, final reduction.
"""

import sys

import numpy as np

sys.path.insert(0, "/opt/trn_rl_repo")

import concourse.bacc as bacc
import concourse.bass as bass
import concourse.tile as tile
from concourse import mybir
from concourse.bass_utils import run_bass_kernel_spmd

B, C, H, W = 64, 2, 512, 512
N_CORES = 8
IMGS = (B // N_CORES) * C  # images per core = 16
F32 = mybir.dt.float32
FP8 = mybir.dt.float8e4
BF16 = mybir.dt.bfloat16

GROUPS = [1, 2, 3, 3, 3, 3, 1]
NG = len(GROUPS)
# w-column split of the squares across ACT / DVE (measured 0.90 / 1.08
# ns/elem incl. overheads); both engines also alternate the V copy.
# engine squares cover w[0:WQTOT] (the PE reduces host-shipped f^2 for
# w[WQTOT:512], except in the last group); split by measured rates
WA, WD = 190, 194
WQTOT = WA + WD  # 384
WQ = W - WQTOT  # 128 columns on the PE
# last group (after the sq-matmul chain stops): engines cover all 512
WAL, WDL = 254, 258

_NC = None


def _pn(n):
    """partitions used by n stripes: image j at partitions 3j..3j+2."""
    return 3 * n


def _build():
    nc = bacc.Bacc()
    flow = nc.declare_dram_parameter("flow", [IMGS, H, W], FP8, isOutput=False)
    # host-squared fp8 of w[384:512], pre-arranged in the SBUF tile layout
    # [p, i, cp, t, w] (rows h = 256*cp + 2p + t)
    flowsq = nc.declare_dram_parameter(
        "flowsq", [128, 2, 2, IMGS, WQ], FP8, isOutput=False
    )
    onesw = nc.declare_dram_parameter("onesw", [128, 2, 16], FP8, isOutput=False)
    # ybasis[k, t, cp, m] = (y at row h=256*cp+2k+t) ** m, fp8-rounded,
    # ybasis[k, j, t, cp, m]: weight set for group-image j, parity t:
    # basis value at column m=3j+a, zeros elsewhere (m padded to 16,
    # the DoubleRow LDWEIGHTS ISA minimum)
    ybas = nc.declare_dram_parameter(
        "ybasis", [128, 3, 2, 2, 16], FP8, isOutput=False
    )
    # v_out[3j+a, g, w] = V[a, image g0+j, w]
    v_out = nc.declare_dram_parameter("v_out", [16, NG, W], BF16, isOutput=True)
    sq_out = nc.declare_dram_parameter("sq_out", [128, 2, NG], F32, isOutput=True)
    sqm_out = nc.declare_dram_parameter("sqm_out", [16, 3 * WQ], BF16, isOutput=True)

    with tile.TileContext(nc) as tc:
        with (
            tc.tile_pool(name="const", bufs=1) as cpool,
            tc.tile_pool(name="img", bufs=7) as ipool,
            tc.tile_pool(name="scr", bufs=2) as spool,
            tc.tile_pool(name="psum", bufs=4, space="PSUM") as ppool,
        ):
            yb = cpool.tile([128, 3, 2, 2, 16], FP8)
            nc.scalar.dma_start(out=yb[:], in_=ybas[:])
            ow = cpool.tile([128, 2, 16], FP8)
            nc.scalar.dma_start(out=ow[:], in_=onesw[:])
            # all groups' PE-side square sums accumulate here; column
            # j*WQ+c collects image-slot j of every group
            psq = pqpool.tile([16, 3 * WQ], F32)
            # dummy Square so the ACT table load (~1.3us) happens in the
            # preamble while ACT is idle, not before the first real square
            warm = cpool.tile([128, 1], F32)
            nc.scalar.activation(
                out=warm[:],
                in_=nc.const_aps.scalar_like(1.0, warm[:]),
                func=mybir.ActivationFunctionType.Square,
            )
            # one tile for both engines' accumulators -> one output DMA
            sq_ad = cpool.tile([128, 2, NG], F32)
            # all groups' V stripes stage here; ONE output DMA at the end
            # (per-group DMAs on the sync queue block later input DMAs)
            v_all = cpool.tile([16, NG, W], BF16)

            g0 = 0
            for g, n in enumerate(GROUPS):
                # img[p, i, cp, t, w]: row h = 256*cp + 2p + t of image i.
                # (t, w) is 1KB contiguous on both sides; (i, cp) merge on
                # the DRAM side -> 3-dim full-rate DMA pattern.
                img = ipool.tile([128, n, 2, 2, W], FP8, tag="img")
                dma_eng = nc.sync if g % 2 == 0 else nc.gpsimd
                dma_eng.dma_start(
                    out=img[:],
                    in_=flow[g0 : g0 + n].rearrange(
                        "i (cp p t) w -> p i cp (t w)", cp=2, p=128, t=2
                    ),
                )

                # V: per (image, parity) one DoubleRow matmul contracting
                # both cp halves; image j's weight set routes its basis to
                # output rows 3j..3j+2, zeros elsewhere, so all 2n matmuls
                # accumulate into one shared (16, W) block.
                psum = ppool.tile([16, W], F32, tag="v")
                for j in range(n):
                    for t in range(2):
                        nc.tensor.matmul(
                            psum[:],
                            yb[:, j, t, :, :],
                            img[:, j, :, t, :],
                            start=(j == 0 and t == 0),
                            stop=(j == n - 1 and t == 1),
                            perf_mode=mybir.MatmulPerfMode.DoubleRow,
                        )

                pe_sq = n == 3 and g <= NG - 2
                if pe_sq:
                    # PE-side squares: ones-weight DoubleRow matmuls reduce
                    # the host-squared stream over h, accumulating across
                    # the n=3 groups in one full-width chain
                    sqi = ipool.tile([128, 2, 2, n, WQ], FP8, tag="sqi")
                    dma_eng.dma_start(
                        out=sqi[:], in_=flowsq[:, :, :, g0 : g0 + n, :]
                    )
                    for t in range(2):
                        nc.tensor.matmul(
                            psq[:],
                            ow[:],
                            sqi[:, :, t, :, :].rearrange("p cp i w -> p cp (i w)"),
                            start=(g == 2 and t == 0),
                            stop=(g == NG - 2 and t == 1),
                            perf_mode=mybir.MatmulPerfMode.DoubleRow,
                        )

                # squares: one pass per engine, hardware accumulators.
                # fp8 scratch: the result stream is discarded, and fp8 out
                # halves SBUF write traffic (the chip power-throttles).
                wa, wd = WA, WD
                scr_a = spool.tile([128, 3, 2, 2, WAL], FP8, tag="scra")
                nc.scalar.activation(
                    out=scr_a[:, :n, :, :, 0:wa],
                    in_=img[:, :, :, :, 0:wa],
                    func=mybir.ActivationFunctionType.Square,
                    accum_out=sq_ad[:, 0, g : g + 1],
                )
                scr_d = spool.tile([128, 3, 2, 2, WDL], FP8, tag="scrd")
                nc.vector.scalar_tensor_tensor(
                    out=scr_d[:, :n, :, :, 0:wd],
                    in0=img[:, :, :, :, wa : wa + wd],
                    scalar=1.0,
                    in1=img[:, :, :, :, wa : wa + wd],
                    op0=mybir.AluOpType.mult,
                    op1=mybir.AluOpType.mult,
                    accum_out=sq_ad[:, 1, g : g + 1],
                )
                if g == NG - 2:
                    # sq chain just stopped: exit it before the last group's
                    # engine squares so the DMA overlaps the tail
                    sqm_sb = cpool.tile([16, 3 * WQ], BF16)
                    nc.scalar.copy(out=sqm_sb[:], in_=psq[:])
                    nc.sync.dma_start(out=sqm_out[:], in_=sqm_sb[:])

                # V exit: one bf16 staging copy over the whole stripe block
                # (free size 512 regardless of n), engines alternating.
                if g % 2 == 0:
                    nc.scalar.copy(out=v_all[:, g, :], in_=psum[:])
                else:
                    nc.vector.tensor_copy(out=v_all[:, g, :], in_=psum[:])
                if g == NG - 1:
                    # exit DMAs, emitted after the last input DMA so their
                    # semaphore waits cannot block it; groups 0..NG-2's V
                    # flushes while the last group still computes
                    nc.sync.dma_start(
                        out=v_out[:, 0 : NG - 1, :], in_=v_all[:, 0 : NG - 1, :]
                    )
                    nc.sync.dma_start(
                        out=v_out[:, NG - 1 :, :], in_=v_all[:, NG - 1 :, :]
                    )
                    nc.sync.dma_start(out=sq_out[:], in_=sq_ad[:])
                g0 += n


    nc.finalize()
    return nc


def _yvals():
    """(q(y), q(y*y)) on the fp8 e4m3 grid, f64."""
    import ml_dtypes

    y = np.linspace(-1.0, 1.0, H, dtype=np.float32)
    qy = y.astype(ml_dtypes.float8_e4m3).astype(np.float64)
    qy2 = (y * y).astype(ml_dtypes.float8_e4m3).astype(np.float64)
    return qy, qy2


def _onesw():
    import ml_dtypes

    O = np.zeros((128, 2, 16), dtype=np.float64)
    O[:, :, 0] = 1.0
    return O.astype(ml_dtypes.float8_e4m3)


def _flowsq(shard8):
    """shard8: (IMGS, H, W) fp8 -> (128, IMGS, 2, 2, WQ) fp8 of f^2 for
    w[WQTOT:], laid out as the SBUF tile [p, i, cp, t, w]."""
    import ml_dtypes

    f2 = (shard8.astype(np.float32) ** 2)[:, :, WQTOT:]
    f2 = f2.reshape(IMGS, 2, 128, 2, WQ)  # i, cp, p, t, w
    return np.ascontiguousarray(
        f2.transpose(2, 1, 3, 0, 4).astype(ml_dtypes.float8_e4m3)
    )


def _ybasis():
    import ml_dtypes

    qy, qy2 = _yvals()
    Y = np.zeros((128, 3, 2, 2, 16), dtype=np.float64)
    for j in range(3):
        for t in range(2):
            for cp in range(2):
                h = 256 * cp + 2 * np.arange(128) + t
                Y[:, j, t, cp, 3 * j + 0] = 1.0
                Y[:, j, t, cp, 3 * j + 1] = qy[h]
                Y[:, j, t, cp, 3 * j + 2] = qy2[h]
    return Y.astype(ml_dtypes.float8_e4m3)


def _gram():
    # G = Phi^T Phi for the basis the device actually applies: y-side on the
    # fp8 grid, x-side exact f64.  Each entry factorizes into y-sum * x-sum.
    qy, qy2 = _yvals()
    yv = [np.ones_like(qy), qy, qy2]
    x = np.linspace(-1.0, 1.0, W, dtype=np.float32).astype(np.float64)
    xv = [np.ones_like(x), x, x * x]
    e = [(0, 0), (0, 1), (1, 0), (0, 2), (1, 1), (2, 0)]
    G = np.empty((6, 6))
    for j in range(6):
        for k in range(6):
            G[j, k] = (yv[e[j][0]] * yv[e[k][0]]).sum() * (
                xv[e[j][1]] * xv[e[k][1]]
            ).sum()
    return G


def _extract_v(v_raw):
    """v_raw: (16, NG, W) -> V (3, IMGS, W) f64."""
    V = np.empty((3, IMGS, W), dtype=np.float64)
    g0 = 0
    for g, n in enumerate(GROUPS):
        for j in range(n):
            for a in range(3):
                V[a, g0 + j, :] = v_raw[3 * j + a, g, :]
        g0 += n
    return V


def _run(shards, ybasis=None, trace=False, **kwargs):
    """shards: (8, IMGS, H, W) float32-or-fp8. Returns BassKernelResults."""
    import ml_dtypes

    global _NC
    if _NC is None:
        _NC = _build()
    if ybasis is None:
        ybasis = _ybasis()
    shards = np.asarray(shards)
    if shards.dtype != ml_dtypes.float8_e4m3:
        shards = shards.astype(ml_dtypes.float8_e4m3)
    in_maps = [
        {"flow": np.ascontiguousarray(shards[k]), "ybasis": ybasis}
        for k in range(N_CORES)
    ]
    return run_bass_kernel_spmd(_NC, in_maps, list(range(N_CORES)), trace=trace, **kwargs)


def kernel(flow_field: np.ndarray) -> np.ndarray:
    import ml_dtypes

    global _NC
    flow = np.asarray(flow_field, dtype=np.float32)
    assert flow.shape == (B, C, H, W)
    shards = np.ascontiguousarray(
        flow.reshape(N_CORES, IMGS, H, W).astype(ml_dtypes.float8_e4m3)
    )

    # rare transient NRT device errors recover on a clean retry
    last_err = None
    for attempt in range(3):
        try:
            res = _run(shards)
            break
        except Exception as e:  # noqa: BLE001
            last_err = e
            _NC = None
    else:
        raise last_err

    G = _gram()
    x = np.linspace(-1.0, 1.0, W, dtype=np.float32).astype(np.float64)
    Xb = np.stack([np.ones_like(x), x, x * x], axis=1)  # (W, 3)

    Ginv = np.linalg.inv(G)
    total = 0.0
    for k in range(N_CORES):
        v = _extract_v(np.asarray(res.results[k]["v_out"], dtype=np.float64))
        sq = np.asarray(res.results[k]["sq_out"], dtype=np.float64)  # (128, 2, NG)
        sqm = np.asarray(res.results[k]["sqm_out"], dtype=np.float64)
        M = np.einsum("aiw,wb->iab", v, Xb)  # (IMGS, 3, 3)
        r = np.stack(
            [M[:, 0, 0], M[:, 0, 1], M[:, 1, 0], M[:, 0, 2], M[:, 1, 1], M[:, 2, 0]],
            axis=1,
        )  # (IMGS, 6)
        fit_energy = np.einsum("ij,jk,ik->i", r, Ginv, r)  # r^T G^-1 r
        total += float(sq.sum() - fit_energy.sum())

    loss = total / (H * W) / B
    return np.asarray(loss, dtype=np.float32)
